# revision 5
# baseline (speedup 1.0000x reference)
"""Trainium2 Bass kernel for segment_reduce (span mean-pool -> entity mean).

Strategy (8 NeuronCores, SPMD, one program + per-core data):
  - Entities are partitioned across the 8 cores (greedy-balanced so per-core
    span-length histograms match); each core owns ~E/8 entities and all of
    their mentions, so no cross-core reduction is needed.
  - enc_seq is replicated to every core; each core gathers only its own
    mentions' span rows from HBM with SWDGE indirect DMA.  Mentions are
    bucketed by span length L so one indirect DMA fetches up to 128 spans of
    exactly L consecutive rows (one descriptor per mention, zero pad traffic).
  - Span sums are computed by log2 free-axis folds on the Vector engine.
  - A one-hot weight matrix W[p, e] = 1/(len_p * cnt_e) built on-chip
    (iota + tensor_scalar is_equal*mult) turns the entity segment-sum into
    PSUM-accumulated matmuls: out[e, :] += sum_p W[p, e] * span_sum[p, :].
  - Per-core output is [E_pc, 256]; the host just re-permutes rows.
"""

import contextlib

import numpy as np

from concourse import bass, mybir
import concourse.tile as tile
from concourse.bass_utils import run_bass_kernel_spmd

# Problem constants (nn_BaseModel_69355131896059)
T, D, M, E, L_MAX = 200000, 256, 20000, 4000, 16
N_CORES = 8
FP32 = mybir.dt.float32
INT32 = mybir.dt.int32

# ---------------------------------------------------------------------------
# Walrus in this container rejects instructions carrying more than ~2 sync
# commands ("Too many sync wait commands").  After Tile scheduling, split
# excess sem waits onto same-engine NOPs inserted before the instruction.
# ---------------------------------------------------------------------------
_WAIT_LIMIT = 1
_nsplit = [0]


def split_excess_waits(nc, limit=_WAIT_LIMIT):
    for fn in nc.m.functions:
        for bb in fn.blocks:
            insts = list(bb.instructions)
            if not any(
                i.sync_info is not None
                and i.sync_info.on_wait
                and len(i.sync_info.on_wait) > limit
                for i in insts
            ):
                continue
            out = []
            for inst in insts:
                si = inst.sync_info
                if si is not None and si.on_wait and len(si.on_wait) > limit:
                    waits = list(si.on_wait)
                    keep, extra = waits[-limit:], waits[:-limit]
                    for s in range(0, len(extra), limit):
                        nop = mybir.InstNoOp(
                            name=f"waitsplit-{_nsplit[0]}",
                            engine=inst.engine,
                            sync_info=mybir.SyncInfo(
                                on_wait=extra[s : s + limit], on_update=[]
                            ),
                        )
                        _nsplit[0] += 1
                        out.append(nop)
                    inst.sync_info = mybir.SyncInfo(
                        on_wait=keep, on_update=list(si.on_update or [])
                    )
                out.append(inst)
            bb.instructions = out


# ---------------------------------------------------------------------------
# Host-side prep: entity->core assignment, length-bucketed mention chunking.
# ---------------------------------------------------------------------------
def _merge_spans(starts, lens):
    """Merge spans into disjoint runs; return (run_lo, run_len, cum) arrays."""
    o = np.argsort(starts, kind="stable")
    s, e = starts[o], starts[o] + lens[o]
    lo, hi, out = [], [], []
    cur_lo, cur_hi = int(s[0]), int(e[0])
    for i in range(1, len(s)):
        if s[i] <= cur_hi:
            cur_hi = max(cur_hi, int(e[i]))
        else:
            out.append((cur_lo, cur_hi))
            cur_lo, cur_hi = int(s[i]), int(e[i])
    out.append((cur_lo, cur_hi))
    run_lo = np.array([a for a, b in out], dtype=np.int64)
    run_len = np.array([b - a for a, b in out], dtype=np.int64)
    cum = np.concatenate([[0], np.cumsum(run_len)])
    return run_lo, run_len, cum


def _host_prep(info, num_entities):
    E_ = int(num_entities)
    eid = np.asarray(info[:, 0], dtype=np.int64)
    starts = np.asarray(info[:, 2], dtype=np.int64)
    ends = np.asarray(info[:, 3], dtype=np.int64)
    lens = ends - starts
    glen = np.minimum(lens, L_MAX)  # reference only pools the first L_MAX rows
    M_ = info.shape[0]

    cnt = np.bincount(eid, minlength=E_).astype(np.float64)
    w_all = 1.0 / (np.maximum(lens, 1) * np.maximum(cnt[eid], 1.0))

    e_pc = -(-E_ // N_CORES)  # entities per core (unpadded)
    e_pc_pad = -(-e_pc // 128) * 128  # padded to 128 for entity tiles

    # mentions grouped per entity
    order = np.argsort(eid, kind="stable")
    ent_start = np.searchsorted(eid[order], np.arange(E_ + 1))

    # per-entity length histograms for greedy balancing
    ent_hist = np.zeros((E_, L_MAX), dtype=np.int64)
    ml = glen[order]
    for e in range(E_):
        seg = ml[ent_start[e] : ent_start[e + 1]]
        if len(seg):
            ent_hist[e] = np.bincount(seg - 1, minlength=L_MAX)
    ent_tot = ent_hist.sum(axis=1)

    # greedy: big entities first, to the core with most bucket headroom
    core_hist = np.zeros((N_CORES, L_MAX), dtype=np.int64)
    core_ents = [[] for _ in range(N_CORES)]
    target = ent_hist.sum(axis=0) / N_CORES
    for e in np.argsort(-ent_tot, kind="stable"):
        best_c, best_score = -1, None
        for c in range(N_CORES):
            if len(core_ents[c]) >= e_pc:
                continue
            over = np.maximum(core_hist[c] + ent_hist[e] - target, 0.0).sum()
            score = (over, len(core_ents[c]))
            if best_score is None or score < best_score:
                best_c, best_score = c, score
        core_ents[best_c].append(e)
        core_hist[best_c] += ent_hist[e]

    # per-core, per-bucket mention lists (entity-local columns)
    #   blists[c][L-1] = list of (start_row, local_entity, weight)
    blists = [[[] for _ in range(L_MAX)] for _ in range(N_CORES)]
    ent_of_core = []
    for c in range(N_CORES):
        ents = np.array(core_ents[c], dtype=np.int64)
        ent_of_core.append(ents)
        for local, e in enumerate(ents):
            for mi in order[ent_start[e] : ent_start[e + 1]]:
                L = int(glen[mi])
                blists[c][L - 1].append((int(starts[mi]), local, float(w_all[mi])))

    # uniform chunk structure across cores: bucket capacity = max count
    caps = [max(len(blists[c][b]) for c in range(N_CORES)) for b in range(L_MAX)]
    chunks = []  # list of (L, P_m) in decreasing-L order
    for b in range(L_MAX - 1, -1, -1):
        K = caps[b]
        while K > 0:
            p = min(128, K)
            chunks.append((b + 1, p))
            K -= p

    n_chunks = len(chunks)
    idx_t = np.zeros((N_CORES, 128, n_chunks), dtype=np.int32)
    ecol_t = np.zeros((N_CORES, 128, n_chunks), dtype=np.float32)
    w_t = np.zeros((N_CORES, 128, n_chunks), dtype=np.float32)
    core_runs = []
    for c in range(N_CORES):
        # compact per-core row table: union of this core's spans, runs merged
        # so every span stays contiguous; remap starts into table coords
        c_starts, c_lens = [], []
        for b in range(L_MAX):
            for s, _, _ in blists[c][b]:
                c_starts.append(s)
                c_lens.append(b + 1)
        c_starts = np.array(c_starts, dtype=np.int64)
        c_lens = np.array(c_lens, dtype=np.int64)
        run_lo, run_len, cum = _merge_spans(c_starts, c_lens)
        core_runs.append((run_lo, run_len, cum))

        def remap(s):
            i = np.searchsorted(run_lo, s, side="right") - 1
            return int(cum[i] + (s - run_lo[i]))

        pos = {b: 0 for b in range(L_MAX)}
        for j, (L, p) in enumerate(chunks):
            b = L - 1
            lst = blists[c][b]
            for q in range(p):
                k = pos[b] + q
                if k < len(lst):
                    s, local, w = lst[k]
                    idx_t[c, q, j] = remap(s)
                    ecol_t[c, q, j] = float(local)
                    w_t[c, q, j] = w
            pos[b] += p

    k_tab = -(-max(int(r[2][-1]) for r in core_runs) // 128) * 128

    return {
        "chunks": chunks,
        "idx": idx_t,
        "ecol": ecol_t,
        "w": w_t,
        "ent_of_core": ent_of_core,
        "e_pc_pad": e_pc_pad,
        "E": E_,
        "core_runs": core_runs,
        "k_tab": k_tab,
    }


def build_tables(enc_np, prep):
    """Gather each core's compacted row table from the full enc_seq."""
    k_tab = prep["k_tab"]
    tabs = []
    for c in range(N_CORES):
        run_lo, run_len, cum = prep["core_runs"][c]
        n = int(cum[-1])
        tab = np.zeros((k_tab, D), dtype=np.float32)
        pos = 0
        for lo, ln in zip(run_lo, run_len):
            tab[pos : pos + ln] = enc_np[lo : lo + ln]
            pos += ln
        tabs.append(tab)
    return tabs


# ---------------------------------------------------------------------------
# Device program
# ---------------------------------------------------------------------------
def build_program(chunks, n_chunks, e_pc_pad, k_tab, n_reps=1, gather_bufs=8):
    nc = bass.Bass("TRN2", target_bir_lowering=False, debug=False,
                   num_devices=N_CORES)
    enc = nc.dram_tensor("enc", [k_tab, D], FP32, kind="ExternalInput").ap()
    idx = nc.dram_tensor("idx", [128, n_chunks], INT32, kind="ExternalInput").ap()
    ecol = nc.dram_tensor("ecol", [128, n_chunks], FP32, kind="ExternalInput").ap()
    wgt = nc.dram_tensor("wgt", [128, n_chunks], FP32, kind="ExternalInput").ap()
    out = nc.dram_tensor("out", [e_pc_pad, D], FP32, kind="ExternalOutput").ap()
    n_etiles = e_pc_pad // 128

    with tile.TileContext(nc) as tc, contextlib.ExitStack() as ctx:
        meta = ctx.enter_context(tc.tile_pool(name="meta", bufs=1))
        gat = ctx.enter_context(tc.tile_pool(name="gat", bufs=gather_bufs))
        wp = ctx.enter_context(tc.tile_pool(name="wp", bufs=4))
        op = ctx.enter_context(tc.tile_pool(name="op", bufs=2))
        pp = ctx.enter_context(tc.tile_pool(name="pp", bufs=1, space="PSUM"))

        idx_sb = meta.tile([128, n_chunks], INT32)
        nc.sync.dma_start(idx_sb[:], idx[:])
        ecol_sb = meta.tile([128, n_chunks], FP32)
        nc.sync.dma_start(ecol_sb[:], ecol[:])
        w_sb = meta.tile([128, n_chunks], FP32)
        nc.sync.dma_start(w_sb[:], wgt[:])
        iota = meta.tile([128, e_pc_pad], FP32)
        nc.gpsimd.iota(iota[:], pattern=[[1, e_pc_pad]], channel_multiplier=0,
                       allow_small_or_imprecise_dtypes=True)

        psums = [
            pp.tile([128, D], FP32, tag=f"ps{t}", name=f"ps{t}")
            for t in range(n_etiles)
        ]

        for rep in range(n_reps):
            for j, (L, Pm) in enumerate(chunks):
                g = gat.tile([128, L_MAX * D], FP32, tag="g", name=f"g_{rep}_{j}")
                nc.gpsimd.indirect_dma_start(
                    out=g[:Pm, : L * D],
                    out_offset=None,
                    in_=enc[:],
                    in_offset=bass.IndirectOffsetOnAxis(
                        ap=idx_sb[:Pm, j : j + 1], axis=0
                    ),
                )
                n = L
                while n > 1:
                    k = n // 2
                    nc.vector.tensor_add(
                        g[:Pm, : k * D],
                        g[:Pm, : k * D],
                        g[:Pm, (n - k) * D : n * D],
                    )
                    n -= k
                W = wp.tile([128, e_pc_pad], FP32, tag="W", name=f"W_{rep}_{j}")
                nc.vector.tensor_scalar(
                    out=W[:Pm, :],
                    in0=iota[:Pm, :],
                    scalar1=ecol_sb[:Pm, j : j + 1],
                    scalar2=w_sb[:Pm, j : j + 1],
                    op0=mybir.AluOpType.is_equal,
                    op1=mybir.AluOpType.mult,
                )
                for t in range(n_etiles):
                    nc.tensor.matmul(
                        out=psums[t][:, :],
                        lhsT=W[:Pm, 128 * t : 128 * (t + 1)],
                        rhs=g[:Pm, :D],
                        start=(j == 0),
                        stop=(j == len(chunks) - 1),
                    )
            for t in range(n_etiles):
                o = op.tile([128, D], FP32, tag="o", name=f"o_{rep}_{t}")
                nc.vector.tensor_copy(o[:], psums[t][:])
                nc.sync.dma_start(out[128 * t : 128 * (t + 1), :], o[:])

    split_excess_waits(nc)
    return nc


# ---------------------------------------------------------------------------
# Public entry point
# ---------------------------------------------------------------------------
def kernel(enc_seq, info, num_entities):
    enc_np = np.ascontiguousarray(np.asarray(enc_seq, dtype=np.float32))
    prep = _host_prep(np.asarray(info), num_entities)
    chunks = prep["chunks"]
    nc = build_program(chunks, len(chunks), prep["e_pc_pad"], prep["k_tab"])

    tabs = build_tables(enc_np, prep)
    in_maps = [
        {
            "enc": tabs[c],
            "idx": np.ascontiguousarray(prep["idx"][c]),
            "ecol": np.ascontiguousarray(prep["ecol"][c]),
            "wgt": np.ascontiguousarray(prep["w"][c]),
        }
        for c in range(N_CORES)
    ]
    r = run_bass_kernel_spmd(nc, in_maps, list(range(N_CORES)))

    E_ = prep["E"]
    entities = np.zeros((E_, D), dtype=np.float32)
    for c in range(N_CORES):
        ents = prep["ent_of_core"][c]
        entities[ents] = r.results[c]["out"][: len(ents)]
    return entities
